# revision 19
# baseline (speedup 1.0000x reference)
import numpy as np

B = 8
SEQ = 4096
D = 1024
N_BASE = 10000.0
N_CORES = 8
SPC = SEQ // N_CORES   # seq rows per core (512)
H = 128                # f32 per 512B unit
UPP = 32               # units per partition per chunk (16KB)
UPC = SPC * D // H     # units per core chunk (4096)

_CACHE = {}


def _compute_pe() -> np.ndarray:
    """Mirror of the reference _pos_encoding (default jax backend, f32)."""
    import jax
    import jax.numpy as jnp

    pos = jnp.arange(SEQ, dtype=jnp.float32)[:, None]
    i = jnp.arange(D // 2, dtype=jnp.float32)
    denom = jnp.power(jnp.float32(N_BASE), 2.0 * i / jnp.float32(D))
    ang = pos / denom
    pe = jnp.stack([jnp.sin(ang), jnp.cos(ang)], axis=-1).reshape(SEQ, D)
    return np.asarray(jax.device_get(pe), dtype=np.float32)


def _pass_dmas(nc, engine, dram, row0, sbuf, u0, nu, to_sbuf, skip15):
    """Move [128 parts x nu units] between the chunk at dram[row0:]
    (natural order: partition p holds units [UPP*p, UPP*p+UPP)) and the
    SBUF region `sbuf` (a tile slice of shape [128, nu, H]), for unit
    columns [u0, u0+nu) of the chunk.

    skip15=False: one [128]-DMA -> 16 engines, nu/4 descs each.
    skip15=True: a [120]-DMA (engines 0-14) + an [8]-DMA (engines 0-7),
    so SDMA engine 15 (~17% slower than 0-14) gets nothing. HWDGE splits
    a DMA's n descriptors into runs of g = smallest divisor of n that is
    >= n/16, assigned to engines 0..n/g-1.
    """
    view = dram[row0 : row0 + 128 * UPP, :].rearrange(
        "(p j) d -> p j d", j=UPP
    )
    us = slice(u0, u0 + nu)
    if not skip15:
        pairs = [(sbuf[:, :, :], view[:, us, :])]
    else:
        pairs = [
            (sbuf[0:120, :, :], view[0:120, us, :]),
            (sbuf[120:128, :, :], view[120:128, us, :]),
        ]
    for sb, dr in pairs:
        if to_sbuf:
            engine.dma_start(out=sb, in_=dr)
        else:
            engine.dma_start(out=dr, in_=sb)


# sub-pass column plans, (u0, nu): chunk 0 ramps in small so the first
# add/write starts early; chunk 7 ramps out small so the final add
# barely delays the last writes. Others move as one full pass.
_PLANS = {
    0: [(0, 16), (16, 16)],
    B - 1: [(0, 16), (16, 16)],
}
_FULL = [(0, 32)]


def _build_program():
    import concourse.bacc as bacc
    import concourse.mybir as mybir
    import concourse.tile as tile

    nc = bacc.Bacc("TRN2")
    f32 = mybir.dt.float32
    x_in = nc.declare_dram_parameter("x", [B * UPC, H], f32, isOutput=False)
    pe_in = nc.declare_dram_parameter("pe", [UPC, H], f32, isOutput=False)
    y_out = nc.declare_dram_parameter("y", [B * UPC, H], f32, isOutput=True)

    with tile.TileContext(nc) as tc:
        with (
            tc.tile_pool(name="pe_pool", bufs=2) as pe_pool,
            tc.tile_pool(name="x_pool", bufs=B - 2) as x_pool,
            tc.tile_pool(name="sub_pool", bufs=4) as sub_pool,
        ):
            # pe halves in separate tiles (independent deps), one per
            # ring, both engine-15-free; peA gates the very first add.
            pe_a = pe_pool.tile([128, 16, H], f32)
            pe_b = pe_pool.tile([128, 16, H], f32)
            _pass_dmas(nc, nc.sync, pe_in, 0, pe_a, 0, 16, True, True)
            _pass_dmas(nc, nc.scalar, pe_in, 0, pe_b, 16, 16, True, True)

            def half_adds(xt, u0, nu):
                # an add operand can only span one pe tile; split the
                # column range at 16 where it crosses the pe_a/pe_b seam
                o = 0
                while nu > 0:
                    t, po = (pe_a, u0) if u0 < 16 else (pe_b, u0 - 16)
                    n = min(nu, 16 - (po % 16)) if u0 < 16 else nu
                    nc.vector.tensor_add(
                        xt[:, o : o + n, :],
                        xt[:, o : o + n, :],
                        t[:, po : po + n, :],
                    )
                    u0 += n
                    o += n
                    nu -= n

            # x sub-passes, each into its own tile for exact deps
            xts = {}
            for b in range(B):
                for u0, nu in _PLANS.get(b, _FULL):
                    pool = x_pool if nu == UPP else sub_pool
                    xt = pool.tile([128, nu, H], f32)
                    _pass_dmas(
                        nc, nc.sync, x_in, b * UPC, xt, u0, nu, True, False
                    )
                    xts[(b, u0)] = xt
            for b in range(B):
                for i, (u0, nu) in enumerate(_PLANS.get(b, _FULL)):
                    xt = xts[(b, u0)]
                    half_adds(xt, u0, nu)
                    # y7's first half is engine-15-free: with pe it
                    # shaves engine 15 to ~86% of the per-engine load,
                    # matching its ~17% lower throughput.
                    ys15 = b == B - 1 and i == 0
                    _pass_dmas(
                        nc, nc.scalar, y_out, b * UPC, xt, u0, nu,
                        False, ys15,
                    )
    if not nc.is_finalized():
        nc.finalize()
    return nc


def _get_state():
    if "nc" not in _CACHE:
        _CACHE["nc"] = _build_program()
    if "pe" not in _CACHE:
        _CACHE["pe"] = _compute_pe()
    return _CACHE["nc"], _CACHE["pe"]


def _in_maps(x, pe):
    in_maps = []
    for c in range(N_CORES):
        xs = np.ascontiguousarray(x[:, c * SPC : (c + 1) * SPC, :]).reshape(
            B * UPC, H
        )
        pes = np.ascontiguousarray(pe[c * SPC : (c + 1) * SPC, :]).reshape(
            UPC, H
        )
        in_maps.append({"x": xs, "pe": pes})
    return in_maps


def kernel(x, seq_len=None, **_):
    from concourse.bass_utils import run_bass_kernel_spmd

    x = np.asarray(x, dtype=np.float32)
    assert x.shape == (B, SEQ, D)
    if seq_len is not None:
        assert int(np.asarray(seq_len)) == SEQ

    nc, pe = _get_state()
    res = run_bass_kernel_spmd(nc, _in_maps(x, pe), list(range(N_CORES))).results

    out = np.empty((B, SEQ, D), dtype=np.float32)
    for c in range(N_CORES):
        out[:, c * SPC : (c + 1) * SPC, :] = res[c]["y"].reshape(B, SPC, D)
    return out


# revision 20
# speedup vs baseline: 1.0155x; 1.0155x over previous
import numpy as np

B = 8
SEQ = 4096
D = 1024
N_BASE = 10000.0
N_CORES = 8
SPC = SEQ // N_CORES   # seq rows per core (512)
H = 128                # f32 per 512B unit
UPP = 32               # units per partition per chunk (16KB)
UPC = SPC * D // H     # units per core chunk (4096)

_CACHE = {}


def _compute_pe() -> np.ndarray:
    """Mirror of the reference _pos_encoding (default jax backend, f32)."""
    import jax
    import jax.numpy as jnp

    pos = jnp.arange(SEQ, dtype=jnp.float32)[:, None]
    i = jnp.arange(D // 2, dtype=jnp.float32)
    denom = jnp.power(jnp.float32(N_BASE), 2.0 * i / jnp.float32(D))
    ang = pos / denom
    pe = jnp.stack([jnp.sin(ang), jnp.cos(ang)], axis=-1).reshape(SEQ, D)
    return np.asarray(jax.device_get(pe), dtype=np.float32)


def _pass_dmas(nc, engine, dram, row0, sbuf, u0, nu, to_sbuf, skip15):
    """Move [128 parts x nu units] between the chunk at dram[row0:]
    (natural order: partition p holds units [UPP*p, UPP*p+UPP)) and the
    SBUF region `sbuf` (a tile slice of shape [128, nu, H]), for unit
    columns [u0, u0+nu) of the chunk.

    skip15=False: one [128]-DMA -> 16 engines, nu/4 descs each.
    skip15=True: a [120]-DMA (engines 0-14) + an [8]-DMA (engines 0-7),
    so SDMA engine 15 (~17% slower than 0-14) gets nothing. HWDGE splits
    a DMA's n descriptors into runs of g = smallest divisor of n that is
    >= n/16, assigned to engines 0..n/g-1.
    """
    view = dram[row0 : row0 + 128 * UPP, :].rearrange(
        "(p j) d -> p j d", j=UPP
    )
    us = slice(u0, u0 + nu)
    if not skip15:
        pairs = [(sbuf[:, :, :], view[:, us, :])]
    else:
        pairs = [
            (sbuf[0:120, :, :], view[0:120, us, :]),
            (sbuf[120:128, :, :], view[120:128, us, :]),
        ]
    for sb, dr in pairs:
        if to_sbuf:
            engine.dma_start(out=sb, in_=dr)
        else:
            engine.dma_start(out=dr, in_=sb)


def _build_program():
    import concourse.bacc as bacc
    import concourse.mybir as mybir
    import concourse.tile as tile

    nc = bacc.Bacc("TRN2")
    f32 = mybir.dt.float32
    x_in = nc.declare_dram_parameter("x", [B * UPC, H], f32, isOutput=False)
    pe_in = nc.declare_dram_parameter("pe", [UPC, H], f32, isOutput=False)
    y_out = nc.declare_dram_parameter("y", [B * UPC, H], f32, isOutput=True)

    hu = UPP // 2
    with tile.TileContext(nc) as tc:
        with (
            tc.tile_pool(name="pe_pool", bufs=1) as pe_pool,
            tc.tile_pool(name="x_pool", bufs=B - 1) as x_pool,
            tc.tile_pool(name="sub_pool", bufs=2) as sub_pool,
        ):
            pe_t = pe_pool.tile([128, UPP, H], f32)
            # pe leads the out-ring (idle until the first add anyway),
            # engine-15-free so the slow engine only carries x/y passes.
            _pass_dmas(nc, nc.scalar, pe_in, 0, pe_t, 0, UPP, True, True)

            # x0-x6: one full 16KB-desc pass each on the sync ring
            xts = []
            for b in range(B - 1):
                xt = x_pool.tile([128, UPP, H], f32)
                _pass_dmas(nc, nc.sync, x_in, b * UPC, xt, 0, UPP, True, False)
                xts.append(xt)
            # x7: two halves so the final add costs 2.3us, not 4.4
            x7h = []
            for hi in range(2):
                xt = sub_pool.tile([128, hu, H], f32)
                _pass_dmas(
                    nc, nc.sync, x_in, (B - 1) * UPC, xt, hi * hu, hu,
                    True, False,
                )
                x7h.append(xt)

            # adds + y passes; y0-y6 on the scalar ring behind pe.
            for b in range(B - 1):
                nc.vector.tensor_add(xts[b][:], xts[b][:], pe_t[:])
                _pass_dmas(
                    nc, nc.scalar, y_out, b * UPC, xts[b], 0, UPP, False, False
                )
            # y7 halves ride the SYNC ring (idle once x is in): both
            # rings drain the output backlog together at the end.
            for hi in range(2):
                sl = slice(hi * hu, (hi + 1) * hu)
                nc.vector.tensor_add(x7h[hi][:], x7h[hi][:], pe_t[:, sl, :])
                _pass_dmas(
                    nc, nc.sync, y_out, (B - 1) * UPC, x7h[hi], hi * hu, hu,
                    False, hi == 0,
                )
    if not nc.is_finalized():
        nc.finalize()
    return nc


def _get_state():
    if "nc" not in _CACHE:
        _CACHE["nc"] = _build_program()
    if "pe" not in _CACHE:
        _CACHE["pe"] = _compute_pe()
    return _CACHE["nc"], _CACHE["pe"]


def _in_maps(x, pe):
    in_maps = []
    for c in range(N_CORES):
        xs = np.ascontiguousarray(x[:, c * SPC : (c + 1) * SPC, :]).reshape(
            B * UPC, H
        )
        pes = np.ascontiguousarray(pe[c * SPC : (c + 1) * SPC, :]).reshape(
            UPC, H
        )
        in_maps.append({"x": xs, "pe": pes})
    return in_maps


def kernel(x, seq_len=None, **_):
    from concourse.bass_utils import run_bass_kernel_spmd

    x = np.asarray(x, dtype=np.float32)
    assert x.shape == (B, SEQ, D)
    if seq_len is not None:
        assert int(np.asarray(seq_len)) == SEQ

    nc, pe = _get_state()
    res = run_bass_kernel_spmd(nc, _in_maps(x, pe), list(range(N_CORES))).results

    out = np.empty((B, SEQ, D), dtype=np.float32)
    for c in range(N_CORES):
        out[:, c * SPC : (c + 1) * SPC, :] = res[c]["y"].reshape(B, SPC, D)
    return out


# revision 21
# speedup vs baseline: 1.0285x; 1.0128x over previous
import numpy as np

B = 8
SEQ = 4096
D = 1024
N_BASE = 10000.0
N_CORES = 8
SPC = SEQ // N_CORES   # seq rows per core (512)
H = 128                # f32 per 512B unit
UPP = 32               # units per partition per chunk (16KB)
UPC = SPC * D // H     # units per core chunk (4096)

_CACHE = {}


def _compute_pe() -> np.ndarray:
    """Mirror of the reference _pos_encoding (default jax backend, f32)."""
    import jax
    import jax.numpy as jnp

    pos = jnp.arange(SEQ, dtype=jnp.float32)[:, None]
    i = jnp.arange(D // 2, dtype=jnp.float32)
    denom = jnp.power(jnp.float32(N_BASE), 2.0 * i / jnp.float32(D))
    ang = pos / denom
    pe = jnp.stack([jnp.sin(ang), jnp.cos(ang)], axis=-1).reshape(SEQ, D)
    return np.asarray(jax.device_get(pe), dtype=np.float32)


def _pass_dmas(nc, engine, dram, row0, tile, u0, nu, to_sbuf, skip15):
    """Move [128 parts x nu units] between the chunk at dram[row0:]
    (natural order: partition p holds units [UPP*p, UPP*p+UPP)) and
    tile[:, u0:u0+nu, :], for unit columns [u0, u0+nu) of the chunk.

    skip15=False: one [128]-DMA -> 16 engines, nu/4 descs each.
    skip15=True: a [120]-DMA (engines 0-14) + an [8]-DMA (engines 0-7),
    so SDMA engine 15 (~17% slower than 0-14) gets nothing. HWDGE splits
    a DMA's n descriptors into runs of g = smallest divisor of n that is
    >= n/16, assigned to engines 0..n/g-1.
    """
    view = dram[row0 : row0 + 128 * UPP, :].rearrange(
        "(p j) d -> p j d", j=UPP
    )
    us = slice(u0, u0 + nu)
    if not skip15:
        pairs = [(tile[:, us, :], view[:, us, :])]
    else:
        pairs = [
            (tile[0:120, us, :], view[0:120, us, :]),
            (tile[120:128, us, :], view[120:128, us, :]),
        ]
    for sb, dr in pairs:
        if to_sbuf:
            engine.dma_start(out=sb, in_=dr)
        else:
            engine.dma_start(out=dr, in_=sb)


# per-chunk sub-pass plans: (u0, nu, x_skip15, y_skip15, y_on_sync)
# chunk 0 ramps in small so the first add/write starts early; chunk 7
# ramps out small (short final adds) with its writes on the otherwise-
# idle sync ring, so both rings drain the output backlog at the end.
_PLANS = {
    0: [
        (0, 8, False, False, False),
        (8, 8, False, False, False),
        (16, 16, False, False, False),
    ],
    B - 1: [
        (0, 16, True, True, True),
        (16, 8, False, True, True),
        (24, 8, False, True, True),
    ],
}
_FULL = [(0, UPP, False, False, False)]


def _build_program():
    import concourse.bacc as bacc
    import concourse.mybir as mybir
    import concourse.tile as tile

    nc = bacc.Bacc("TRN2")
    f32 = mybir.dt.float32
    x_in = nc.declare_dram_parameter("x", [B * UPC, H], f32, isOutput=False)
    pe_in = nc.declare_dram_parameter("pe", [UPC, H], f32, isOutput=False)
    y_out = nc.declare_dram_parameter("y", [B * UPC, H], f32, isOutput=True)

    with tile.TileContext(nc) as tc:
        with (
            tc.tile_pool(name="pe_pool", bufs=1) as pe_pool,
            tc.tile_pool(name="x_pool", bufs=B) as x_pool,
        ):
            pe_t = pe_pool.tile([128, UPP, H], f32)
            # pe halves ride both rings up front, engine-15-free: with
            # y7's first half they shave engine 15 to match its lower
            # mixed-regime throughput.
            _pass_dmas(nc, nc.sync, pe_in, 0, pe_t, 0, 16, True, True)
            _pass_dmas(nc, nc.scalar, pe_in, 0, pe_t, 16, 16, True, True)
            xts = []
            for b in range(B):
                xt = x_pool.tile([128, UPP, H], f32)
                for u0, nu, xs15, _ys15, _ysync in _PLANS.get(b, _FULL):
                    _pass_dmas(
                        nc, nc.sync, x_in, b * UPC, xt, u0, nu, True, xs15
                    )
                xts.append(xt)
            for b in range(B):
                for u0, nu, _xs15, ys15, ysync in _PLANS.get(b, _FULL):
                    sl = slice(u0, u0 + nu)
                    nc.vector.tensor_add(
                        xts[b][:, sl, :], xts[b][:, sl, :], pe_t[:, sl, :]
                    )
                    eng = nc.sync if ysync else nc.scalar
                    _pass_dmas(
                        nc, eng, y_out, b * UPC, xts[b], u0, nu, False, ys15
                    )
    if not nc.is_finalized():
        nc.finalize()
    return nc


def _get_state():
    if "nc" not in _CACHE:
        _CACHE["nc"] = _build_program()
    if "pe" not in _CACHE:
        _CACHE["pe"] = _compute_pe()
    return _CACHE["nc"], _CACHE["pe"]


def _in_maps(x, pe):
    in_maps = []
    for c in range(N_CORES):
        xs = np.ascontiguousarray(x[:, c * SPC : (c + 1) * SPC, :]).reshape(
            B * UPC, H
        )
        pes = np.ascontiguousarray(pe[c * SPC : (c + 1) * SPC, :]).reshape(
            UPC, H
        )
        in_maps.append({"x": xs, "pe": pes})
    return in_maps


def kernel(x, seq_len=None, **_):
    from concourse.bass_utils import run_bass_kernel_spmd

    x = np.asarray(x, dtype=np.float32)
    assert x.shape == (B, SEQ, D)
    if seq_len is not None:
        assert int(np.asarray(seq_len)) == SEQ

    nc, pe = _get_state()
    res = run_bass_kernel_spmd(nc, _in_maps(x, pe), list(range(N_CORES))).results

    out = np.empty((B, SEQ, D), dtype=np.float32)
    for c in range(N_CORES):
        out[:, c * SPC : (c + 1) * SPC, :] = res[c]["y"].reshape(B, SPC, D)
    return out
